# revision 1
# baseline (speedup 1.0000x reference)
"""Trainium2 Bass kernel for nn_BilinearModule (16,256,64,64 bilinear pooling).

Math (per image):
  y   = relu(bn1(w1 @ x + b1))                       # (32, 4096)
  packed[t] = y[r_t] * y[c_t]  for 528 lower-tri pairs
  out = relu(bn2(w2 @ packed + b2))                  # (256, 4096)

Strategy (pure data parallel over batch, 2 images per core, 8 cores):
  - mm1 in f32r with M-replicated weights -> psum; fused BN1+ReLU on ACT -> yrep
    (4 identical copies of the 32 channels across 128 partitions).
  - The 528 pair-products are covered by 17 channel *rotations* r=0..16:
    rotation r yields pairs {c, (c+r)%32} which is diag r plus diag 32-r,
    all distinct (r=16 half-duplicated). Rotated tiles are produced by 5
    K=32 permutation matmuls (4 rotations per tile, one per quadrant),
    issued at different PE row-strips so they overlap in the array.
  - Products on DVE/GpSimd (f32r), second matmul = 5 K=128 f32r chunks with
    host-side permuted+zero-padded w2; fused BN2+ReLU on ACT.
All weights are preprocessed host-side; pair order is folded into w2.
"""

import numpy as np

import concourse.bass as bass
import concourse.mybir as mybir
from concourse import tile
from concourse.bass_utils import run_bass_kernel_spmd

F32 = mybir.dt.float32
F32R = mybir.dt.float32r
AF = mybir.ActivationFunctionType

N_CORES = 8
B, CIN, H, W = 16, 256, 64, 64
NPIX = H * W                     # 4096
IMG_PER_CORE = B // N_CORES      # 2
CMID = 32
COUT = 256
FB = 512                         # pixel window (psum-bank sized)
NWIN = NPIX // FB                # 8 windows per image
EPS = 1e-5

# rotation sets per product tile (quadrant q of tile j uses ROTS[j][q])
ROTS = [[0, 1, 2, 3], [4, 5, 6, 7], [8, 9, 10, 11], [12, 13, 14, 15], [16, 0, 0, 0]]

_ctr = [0]


def _split_multi_waits(nc):
    """This container's walrus supports one sync-wait per instruction; split
    extras onto NOP carriers on the same engine."""
    for f in nc.m.functions:
        for blk in f.blocks:
            insts = blk.instructions
            if not any(
                i.sync_info is not None and len(i.sync_info.on_wait) > 1
                for i in insts
            ):
                continue
            new = []
            for inst in insts:
                si = inst.sync_info
                if si is not None and len(si.on_wait) > 1:
                    waits = list(si.on_wait)
                    for wcond in waits[:-1]:
                        _ctr[0] += 1
                        nop = mybir.InstNoOp(name=f"waitnop-{_ctr[0]}", ins=[], outs=[])
                        nop.engine = inst.engine
                        nop.sync_info = mybir.SyncInfo(on_wait=[wcond], on_update=[])
                        new.append(nop)
                    inst.sync_info = mybir.SyncInfo(
                        on_wait=[waits[-1]], on_update=list(si.on_update)
                    )
                new.append(inst)
            blk.instructions = new


def _host_weights(w1, b1, g1, be1, m1, v1, w2, b2, g2, be2, m2, v2):
    """Precompute device weight layouts on the host."""
    # mm1 lhsT, M-replicated: w1t[k, 32q+c] = w1[c, k]
    w1t = np.zeros((CIN, 128), np.float32)
    for q in range(4):
        w1t[:, 32 * q : 32 * q + 32] = w1.T
    inv1 = g1 / np.sqrt(v1 + EPS)
    bn1s = np.tile(inv1, 4).reshape(128, 1).astype(np.float32)
    bn1b = np.tile(b1 * inv1 + be1 - m1 * inv1, 4).reshape(128, 1).astype(np.float32)

    # permutation lhsT for the 5 rotation tiles, replicated across 4 strips:
    # perm[32i + k, 128j + 32q + c] = 1 iff k == (c + ROTS[j][q]) % 32
    perm = np.zeros((128, 5 * 128), np.float32)
    for j in range(5):
        for q in range(4):
            r = ROTS[j][q]
            for c in range(32):
                k = (c + r) % 32
                for i in range(4):
                    perm[32 * i + k, 128 * j + 32 * q + c] = 1.0

    # w2 permuted into the 5x128 product-row order; duplicate slots zeroed.
    off = np.zeros(33, np.int64)
    for d in range(32):
        off[d + 1] = off[d] + (32 - d)
    assert off[32] == 528
    w2p = np.zeros((5 * 128, COUT), np.float32)
    used = np.zeros(528, bool)
    for j in range(5):
        for q in range(4):
            r = ROTS[j][q]
            if j == 4 and q > 0:
                continue  # spare quadrants: weights stay zero
            for c in range(32):
                if r == 16 and c >= 16:
                    continue  # duplicate half of rotation 16
                if c + r < 32:
                    d, b_lo = r, c
                else:
                    d, b_lo = 32 - r, c + r - 32
                t = off[d] + b_lo
                assert not used[t]
                used[t] = True
                w2p[128 * j + 32 * q + c, :] = w2[:, t]
    assert used.all()

    inv2 = g2 / np.sqrt(v2 + EPS)
    bn2s = inv2.reshape(2, 128).T.astype(np.float32).copy()   # [128, 2] col m
    bn2b = (b2 * inv2 + be2 - m2 * inv2).reshape(2, 128).T.astype(np.float32).copy()
    return w1t, bn1s, bn1b, perm, w2p, bn2s, bn2b


def _build_nc():
    nc = bass.Bass()
    x_d = nc.declare_dram_parameter("x", [IMG_PER_CORE, CIN, NPIX], F32R, isOutput=False)
    w1t_d = nc.declare_dram_parameter("w1t", [CIN, 128], F32R, isOutput=False)
    bn1s_d = nc.declare_dram_parameter("bn1s", [128, 1], F32, isOutput=False)
    bn1b_d = nc.declare_dram_parameter("bn1b", [128, 1], F32, isOutput=False)
    perm_d = nc.declare_dram_parameter("perm", [128, 5 * 128], F32R, isOutput=False)
    w2p_d = nc.declare_dram_parameter("w2p", [5 * 128, COUT], F32R, isOutput=False)
    bn2s_d = nc.declare_dram_parameter("bn2s", [128, 2], F32, isOutput=False)
    bn2b_d = nc.declare_dram_parameter("bn2b", [128, 2], F32, isOutput=False)
    out_d = nc.declare_dram_parameter("out", [IMG_PER_CORE, COUT, NPIX], F32, isOutput=True)

    with tile.TileContext(nc) as tc:
        with (
            tc.tile_pool(name="consts", bufs=1) as cpool,
            tc.tile_pool(name="xp", bufs=2) as xpool,
            tc.tile_pool(name="yp", bufs=3) as ypool,
            tc.tile_pool(name="y4p", bufs=4) as y4pool,
            tc.tile_pool(name="pp", bufs=7) as ppool,
            tc.tile_pool(name="zp", bufs=3) as zpool,
            tc.tile_pool(name="psy", bufs=2, space="PSUM") as psum_y,
            tc.tile_pool(name="pss", bufs=3, space="PSUM") as psum_sel,
            tc.tile_pool(name="psz", bufs=2, space="PSUM") as psum_z,
        ):
            w1a = cpool.tile([128, 128], F32R, tag="w1a")
            w1b = cpool.tile([128, 128], F32R, tag="w1b")
            nc.sync.dma_start(w1a[:], w1t_d[0:128, :])
            nc.sync.dma_start(w1b[:], w1t_d[128:256, :])
            perm_sb = cpool.tile([128, 5 * 128], F32R, tag="perm")
            nc.sync.dma_start(perm_sb[:], perm_d[:])
            w2p_sb = cpool.tile([128, 5 * COUT], F32R, tag="w2p")
            for j in range(5):
                nc.sync.dma_start(
                    w2p_sb[:, j * COUT : (j + 1) * COUT],
                    w2p_d[j * 128 : (j + 1) * 128, :],
                )
            bn1s = cpool.tile([128, 1], F32, tag="bn1s")
            bn1b = cpool.tile([128, 1], F32, tag="bn1b")
            bn2s = cpool.tile([128, 2], F32, tag="bn2s")
            bn2b = cpool.tile([128, 2], F32, tag="bn2b")
            nc.sync.dma_start(bn1s[:], bn1s_d[:])
            nc.sync.dma_start(bn1b[:], bn1b_d[:])
            nc.sync.dma_start(bn2s[:], bn2s_d[:])
            nc.sync.dma_start(bn2b[:], bn2b_d[:])

            for img in range(IMG_PER_CORE):
                xa = xpool.tile([128, NPIX], F32R, tag="xa")
                xb = xpool.tile([128, NPIX], F32R, tag="xb")
                nc.sync.dma_start(xa[:], x_d[img, 0:128, :])
                nc.sync.dma_start(xb[:], x_d[img, 128:256, :])

                for win in range(NWIN):
                    s = slice(win * FB, (win + 1) * FB)
                    ps_y = psum_y.tile([128, FB], F32, tag="psy")
                    nc.tensor.matmul(ps_y[:], w1a[:], xa[:, s], start=True, stop=False)
                    nc.tensor.matmul(ps_y[:], w1b[:], xb[:, s], start=False, stop=True)
                    yrep = ypool.tile([128, FB], F32R, tag="yrep")
                    nc.scalar.activation(
                        yrep[:], ps_y[:], AF.Relu, bias=bn1b[:, 0:1], scale=bn1s[:, 0:1]
                    )

                    prods = []
                    for j in range(5):
                        i = j % 4
                        ps_sel = psum_sel.tile([128, FB], F32, tag="pssel")
                        nc.tensor.matmul(
                            ps_sel[:],
                            perm_sb[32 * i : 32 * i + 32, 128 * j : 128 * (j + 1)],
                            yrep[32 * i : 32 * i + 32, :],
                            start=True,
                            stop=True,
                            tile_position=(32 * i, 0),
                        )
                        y4 = y4pool.tile([128, FB], F32R, tag="y4")
                        if j in (1, 3):
                            nc.scalar.activation(y4[:], ps_sel[:], AF.Copy)
                        else:
                            nc.vector.tensor_copy(y4[:], ps_sel[:])
                        pj = ppool.tile([128, FB], F32R, tag="pj")
                        if j in (0, 1):
                            nc.gpsimd.tensor_mul(pj[:], yrep[:], y4[:])
                        else:
                            nc.vector.tensor_mul(pj[:], yrep[:], y4[:])
                        prods.append(pj)

                    for m in range(2):
                        ps_z = psum_z.tile([128, FB], F32, tag="psz")
                        for j in range(5):
                            nc.tensor.matmul(
                                ps_z[:],
                                w2p_sb[:, j * COUT + 128 * m : j * COUT + 128 * m + 128],
                                prods[j][:],
                                start=(j == 0),
                                stop=(j == 4),
                            )
                        zt = zpool.tile([128, FB], F32, tag="zt")
                        nc.scalar.activation(
                            zt[:], ps_z[:], AF.Relu,
                            bias=bn2b[:, m : m + 1], scale=bn2s[:, m : m + 1],
                        )
                        nc.sync.dma_start(out_d[img, 128 * m : 128 * m + 128, s], zt[:])

    _split_multi_waits(nc)
    return nc


_cached = {}


def kernel(**inputs):
    x = np.ascontiguousarray(np.asarray(inputs["x"], np.float32))
    args = [
        np.asarray(inputs[k], np.float32)
        for k in ("w1", "b1", "g1", "be1", "m1", "v1", "w2", "b2", "g2", "be2", "m2", "v2")
    ]
    w1t, bn1s, bn1b, perm, w2p, bn2s, bn2b = _host_weights(*args)

    if "nc" not in _cached:
        _cached["nc"] = _build_nc()
    nc = _cached["nc"]

    xr = x.reshape(B, CIN, NPIX)
    shared = {
        "w1t": w1t, "bn1s": bn1s, "bn1b": bn1b, "perm": perm,
        "w2p": w2p, "bn2s": bn2s, "bn2b": bn2b,
    }
    in_maps = [
        {"x": np.ascontiguousarray(xr[c * IMG_PER_CORE : (c + 1) * IMG_PER_CORE]), **shared}
        for c in range(N_CORES)
    ]
    res = run_bass_kernel_spmd(nc, in_maps, core_ids=list(range(N_CORES)))
    kernel.last_results = res
    out = np.concatenate([res.results[c]["out"] for c in range(N_CORES)], axis=0)
    return out.reshape(B, COUT, H, W)
